# revision 1
# baseline (speedup 1.0000x reference)
"""Multi-head attention (B=4, L=1024, D=1024, H=16) on 8 TRN2 NeuronCores.

Sharding: pure data-parallel over (batch, query-half) — core c handles batch
c//2, query rows [512*(c%2), 512*(c%2+1)). Each core computes Q/K/V
projections for its batch (K/V duplicated across the 2 cores of a batch),
full attention for its 512 queries, and the output projection for its slice.
No collectives; the host concatenates the 8 output slices.

Everything on-device is kept in transposed layouts so no transposes are
needed anywhere:
  Q^T[vd, q]  = Wq(lhsT) @ qT(rhs)         (+bq per-partition via DVE)
  K^T[vd, k]  = Wk(lhsT) @ xT(rhs)         (+bk per-partition)
  V  [k, vd]  = xT(lhsT) @ Wv(rhs)         (+bv via K=1 ones-row matmul)
  S^T[k, q]   = K^T_h(lhsT, K=64) @ Q^T_h  for a head PAIR into one
                [128, 2, 512] 2-bank PSUM tile (heads 2j / 2j+1 at the two
                PE row groups), one exp per k-tile
  expS        = exp(S^T/8 + kmask_bias)    (ScalarE, PSUM->SBUF bf16)
  O^T+denom   = V_aug(lhsT, M=65) @ expS   (V cols + ones col per head)
  scale       = 1/denom broadcast to 64 partitions via K=1 ones matmul
  out[q, d]   = (O^T_scaled.T @ Wo) * q_mask + bo   (DVE epilogue)
"""

import os

os.environ.setdefault("MYCRO_LOCAL_CACHE", "1")

import numpy as np
import ml_dtypes

BF16 = ml_dtypes.bfloat16

B, LQ, LK = 4, 1024, 1024
D = 1024  # QD = KD = VD
H, DH = 16, 64
QS = 512  # queries per core
NCORES = 8
NEG = -1e4  # additive key-mask bias (exp(-1e4) == 0 in f32)

_NC_CACHE = {}


def _build_nc():
    import concourse.bacc as bacc
    import concourse.mybir as mybir
    import concourse.tile as tile

    dt = mybir.dt

    nc = bacc.Bacc(
        "TRN2",
        debug=False,
        target_bir_lowering=False,
        num_devices=NCORES,
    )

    def din(name, shape, dtype):
        return nc.dram_tensor(name, shape, dtype, kind="ExternalInput").ap()

    aps = {
        "qT": din("qT", [D, QS], dt.bfloat16),
        "xT": din("xT", [D, LK], dt.bfloat16),
        "Wq": din("Wq", [D, D], dt.bfloat16),
        "Wk": din("Wk", [D, D], dt.bfloat16),
        "Wv": din("Wv", [D, D], dt.bfloat16),
        "Wo": din("Wo", [D, D], dt.bfloat16),
        # packed per-partition constants: cols 0-7 bq, 8-15 bk, 16-23 kbias,
        # 24-27 q_mask (by query tile)
        "consts": din("consts", [128, 28], dt.float32),
        "bvr": din("bvr", [1, D], dt.bfloat16),
        "bor": din("bor", [1, D], dt.bfloat16),
        "out": nc.dram_tensor("out", [QS, D], dt.float32,
                              kind="ExternalOutput").ap(),
    }

    with tile.TileContext(nc) as tc:
        _body(tc, dt, mybir, aps)

    nc.compile()
    return nc


def _body(tc, dt, mybir, aps):
    from contextlib import ExitStack

    ALU = mybir.AluOpType
    AF = mybir.ActivationFunctionType
    nc = tc.nc
    with ExitStack() as ctx:
        const = ctx.enter_context(tc.tile_pool(name="const", bufs=1))
        espool = ctx.enter_context(tc.tile_pool(name="es", bufs=12))
        psum = ctx.enter_context(tc.tile_pool(name="psum", bufs=4, space="PSUM"))
        spair = ctx.enter_context(tc.tile_pool(name="spair", bufs=2, space="PSUM"))
        opool = ctx.enter_context(tc.tile_pool(name="osb", bufs=3))
        srpool = ctx.enter_context(tc.tile_pool(name="srp", bufs=2))

        def ctile(shape, dtype, tag):
            return const.tile(shape, dtype, tag=tag, name=tag)

        # ---- inputs: halved big strided DMAs, ordered by first use ----
        cst = ctile([128, 28], dt.float32, "cst")
        nc.sync.dma_start(cst[:], aps["consts"][:, :])
        bv_sb = ctile([1, D], dt.bfloat16, "bv")
        nc.scalar.dma_start(bv_sb[:], aps["bvr"][:, :])
        bo_sb = ctile([1, D], dt.bfloat16, "bo")
        nc.scalar.dma_start(bo_sb[:], aps["bor"][:, :])

        halves = {}
        engs = [nc.sync, nc.scalar, nc.gpsimd]
        ei = [0]

        def load_chunked(name, dram, nfree, nchunks):
            per = 8 // nchunks
            tiles = []
            view = dram.rearrange("(t p) n -> p t n", p=128)
            for ch in range(nchunks):
                tl = ctile([128, per, nfree], dt.bfloat16, f"{name}_{ch}")
                engs[ei[0] % 3].dma_start(
                    tl[:, :, :], view[:, per * ch:per * (ch + 1), :])
                ei[0] += 1
                tiles.append(tl)
            halves[name] = (tiles, per)

        def tile_of(name, t):
            tiles, per = halves[name]
            return tiles[t // per][:, t % per, :]

        load_chunked("qT", aps["qT"], QS, 2)
        load_chunked("wq", aps["Wq"], D, 4)
        load_chunked("xT", aps["xT"], LK, 4)
        load_chunked("wk", aps["Wk"], D, 4)
        load_chunked("wv", aps["Wv"], D, 2)
        load_chunked("wo", aps["Wo"], D, 2)

        bq_c = lambda j: cst[:, j:j + 1]
        bk_c = lambda j: cst[:, 8 + j:9 + j]
        kb_c = lambda kt: cst[:, 16 + kt:17 + kt]
        qm_c = lambda qt: cst[:, 24 + qt:25 + qt]

        ones1 = ctile([1, 128], dt.bfloat16, "ones1")
        nc.gpsimd.memset(ones1[:], 1.0)
        ones64 = ctile([1, 64], dt.bfloat16, "ones64")
        nc.gpsimd.memset(ones64[:], 1.0)

        # bo broadcast to all partitions (final tiles add it with DVE)
        bo_rep = ctile([128, D], dt.float32, "bo_rep")
        for n in range(2):
            c = slice(512 * n, 512 * (n + 1))
            ps = psum.tile([128, 512], dt.float32, tag="ps", name="ps")
            nc.tensor.matmul(ps[:], ones1[:], bo_sb[:, c], start=True, stop=True)
            nc.vector.tensor_copy(bo_rep[:, c], ps[:])

        # ---- Q^T projection (first: needs only qT+Wq, ~3 MB) ----
        qTp = [ctile([128, QS], dt.bfloat16, f"qTp{j}") for j in range(8)]
        for j in range(8):
            ps = psum.tile([128, QS], dt.float32, tag="ps", name="ps")
            for kt in range(8):
                nc.tensor.matmul(
                    ps[:], tile_of("wq", kt)[:, 128 * j:128 * (j + 1)],
                    tile_of("qT", kt)[:], start=(kt == 0), stop=(kt == 7))
            nc.vector.tensor_scalar_add(qTp[j][:], ps[:], bq_c(j))

        # ---- K^T projection, vd-tile j = heads (2j, 2j+1) ----
        kT_sb = [ctile([128, LK], dt.bfloat16, f"kT{j}") for j in range(8)]

        def k_proj(j):
            for n in range(2):
                c = slice(512 * n, 512 * (n + 1))
                ps = psum.tile([128, 512], dt.float32, tag="ps", name="ps")
                for kt in range(8):
                    nc.tensor.matmul(
                        ps[:], tile_of("wk", kt)[:, 128 * j:128 * (j + 1)],
                        tile_of("xT", kt)[:, c], start=(kt == 0), stop=(kt == 7))
                nc.vector.tensor_scalar_add(kT_sb[j][:, c], ps[:], bk_c(j))

        k_proj(0)
        k_proj(1)

        # ---- V projection into V_aug layout: per k-tile [128, 16*(64+1)],
        # head h at cols [65h, 65h+64), ones at col 65h+64. Tiles 3-7 are
        # traced INSIDE pair 0's attention stream (3 steps before their
        # o_stage consumer) so the exp pipeline starts ~40us earlier instead
        # of waiting behind the whole V projection in the in-order PE queue.
        v_sb = [ctile([128, H * (DH + 1)], dt.bfloat16, f"v{t}") for t in range(8)]
        for t in range(8):
            ones_cols = v_sb[t][:].rearrange(
                "p (h c) -> p h c", c=DH + 1)[:, :, DH:DH + 1]
            nc.gpsimd.memset(ones_cols, 1.0)

        def v_proj(t):
            for n in range(2):
                c = slice(512 * n, 512 * (n + 1))
                ps = psum.tile([128, 512], dt.float32, tag="ps", name="ps")
                for kd in range(8):
                    nc.tensor.matmul(
                        ps[:], tile_of("xT", kd)[:, 128 * t:128 * (t + 1)],
                        tile_of("wv", kd)[:, c], start=(kd == 0), stop=False)
                nc.tensor.matmul(ps[:], ones1[:], bv_sb[:, c],
                                 start=False, stop=True)
                # one strided cast: psum [p, 8, 64] -> v_aug cols, stride 65
                vout = v_sb[t][:].rearrange(
                    "p (h c) -> p h c", c=DH + 1)[:, 8 * n:8 * n + 8, 0:DH]
                vin = ps[:].rearrange("p (i c) -> p i c", c=DH)
                nc.vector.tensor_copy(vout, vin)

        for t in range(3):
            v_proj(t)

        # ---- attention, one head-pair (2j, 2j+1) at a time; S/exp one
        # k-tile ahead of the O accumulation so the PE never waits on exp ----
        oTs = [ctile([128, QS], dt.bfloat16, f"oTs{j}") for j in range(8)]
        dscr = ctile([1, 2 * QS], dt.float32, "dscr")
        sca = ctile([1, 2 * QS], dt.float32, "sca")
        scb = ctile([1, 2 * QS], dt.bfloat16, "scb")
        rscr = ctile([1, 2 * QS], dt.float32, "rscr")

        es_tiles = {}  # (j, kt) -> es tile

        def s_stage(j, kt):
            kc = slice(128 * kt, 128 * (kt + 1))
            sp = spair.tile([128, 2, QS], dt.float32, tag="sp", name="sp")
            nc.tensor.matmul(sp[:, 0, :], kT_sb[j][0:64, kc],
                             qTp[j][0:64, :], start=True, stop=True)
            nc.tensor.matmul(sp[:, 1, :], kT_sb[j][64:128, kc],
                             qTp[j][64:128, :], start=True, stop=True)
            es = espool.tile([128, 2, QS], dt.bfloat16, tag="es", name="es")
            nc.scalar.activation(es[:], sp[:], AF.Exp,
                                 bias=kb_c(kt), scale=0.125)
            es_tiles[(j, kt)] = es

        def o_stage(j, kt, oA, oB):
            hA, hB = 2 * j, 2 * j + 1
            es = es_tiles.pop((j, kt))
            nc.tensor.matmul(oA[0:65, :], v_sb[kt][:, 65 * hA:65 * hA + 65],
                             es[:, 0, :], start=(kt == 0), stop=(kt == 7))
            nc.tensor.matmul(oB[0:65, :], v_sb[kt][:, 65 * hB:65 * hB + 65],
                             es[:, 1, :], start=(kt == 0), stop=(kt == 7))

        def o_alloc():
            oA = psum.tile([128, QS], dt.float32, tag="ps", name="ps")
            oB = psum.tile([128, QS], dt.float32, tag="ps", name="ps")
            return oA, oB

        # flat software-pipelined stream: S/exp stages run LOOKAHEAD stages
        # ahead of the O accumulation (even across pair boundaries) so the
        # ScalarE exp pipeline never drains — it is the attention-phase
        # bottleneck and can start during the V projection
        LOOKAHEAD = 8
        stages = [(j, kt) for j in range(8) for kt in range(8)]
        s_cursor = [0]

        def advance_s(upto):
            while s_cursor[0] < min(upto, 64):
                s_stage(*stages[s_cursor[0]])
                s_cursor[0] += 1

        fps_early = [None, None]
        cur = o_alloc()
        advance_s(1)
        for j in range(8):
            oA, oB = cur
            for kt in range(1, 8):
                advance_s(8 * j + kt + LOOKAHEAD)
                o_stage(j, kt - 1, oA, oB)
                if j == 0 and kt + 2 < 8:
                    v_proj(kt + 2)
            if j < 7:
                cur = o_alloc()
            advance_s(8 * j + 8 + LOOKAHEAD)
            o_stage(j, 7, oA, oB)

            # row 64 = denominator. Free oA/oB quickly (copy to packed bf16
            # oTu) so the next pair's O accumulation gets PSUM banks while
            # the scale chain runs (DVE); K-proj matmuls are traced before
            # the sr matmuls so the in-order PE queue has filler work while
            # the reciprocal chain completes.
            nc.vector.tensor_copy(dscr[0:1, 0:QS], oA[64:65, :])
            nc.vector.tensor_copy(dscr[0:1, QS:2 * QS], oB[64:65, :])
            oTu = srpool.tile([128, QS], dt.bfloat16, tag="oTu", name="oTu")
            nc.vector.tensor_copy(oTu[0:64, :], oA[0:64, :])
            nc.vector.tensor_copy(oTu[64:128, :], oB[0:64, :])
            nc.vector.reciprocal_approx_accurate(out=sca[:], in_=dscr[:],
                                                 scratch=rscr[:])
            nc.vector.tensor_copy(scb[:], sca[:])
            # PE filler while the DVE reciprocal chain runs (the sr matmuls
            # below sit in the in-order PE queue behind it): k_proj for a
            # later pair, or for the last two pairs a partial accumulation
            # (j=0..5) of the first two output-projection tiles.
            if j + 2 < 8:
                k_proj(j + 2)
            else:
                fe = psum.tile([128, 512], dt.float32, tag="ps", name="ps")
                fc = slice(512 * (j - 6), 512 * (j - 5))
                for jj in range(6):
                    nc.tensor.matmul(fe[:], oTs[jj][:, 0:128],
                                     tile_of("wo", jj)[:, fc],
                                     start=(jj == 0), stop=False)
                fps_early[j - 6] = fe
                if j == 7:
                    # a third early tile fits PSUM here (oA/oB released by
                    # the oTu copies above): (qt1, n0) through j=6. A 4th
                    # would deadlock: sr below needs the last free slot.
                    fe2 = psum.tile([128, 512], dt.float32, tag="ps",
                                    name="ps")
                    for jj in range(7):
                        nc.tensor.matmul(fe2[:], oTs[jj][:, 128:256],
                                         tile_of("wo", jj)[:, 0:512],
                                         start=(jj == 0), stop=False)
                    fps_early.append(fe2)
                    # oTs[6] is ready now: extend the held qt0 tiles to j=6
                    nc.tensor.matmul(fps_early[0][:], oTs[6][:, 0:128],
                                     tile_of("wo", 6)[:, 0:512],
                                     start=False, stop=False)
                    nc.tensor.matmul(fps_early[1][:], oTs[6][:, 0:128],
                                     tile_of("wo", 6)[:, 512:1024],
                                     start=False, stop=False)
            sr = psum.tile([128, QS], dt.float32, tag="ps", name="ps")
            nc.tensor.matmul(sr[0:64, :], ones64[:], scb[:, 0:QS],
                             start=True, stop=True)
            nc.tensor.matmul(sr[64:128, :], ones64[:], scb[:, QS:2 * QS],
                             start=True, stop=True, tile_position=(0, 64))
            nc.vector.tensor_mul(oTs[j][:], oTu[:], sr[:])

        # ---- output projection: out[q, d] = (O^T.T @ Wo) * q_mask + bo ----
        for qt in (1, 2, 3, 0):  # pair-7-dependent resumes last
            qr = slice(128 * qt, 128 * (qt + 1))
            for n in range(2):
                c = slice(512 * n, 512 * (n + 1))
                if qt == 0 and fps_early[n] is not None:
                    ps = fps_early[n]
                    j0 = 7
                elif qt == 1 and n == 0 and len(fps_early) > 2:
                    ps = fps_early[2]
                    j0 = 7
                else:
                    ps = psum.tile([128, 512], dt.float32, tag="ps", name="ps")
                    j0 = 0
                for j in range(j0, 8):
                    nc.tensor.matmul(ps[:], oTs[j][:, qr],
                                     tile_of("wo", j)[:, c],
                                     start=(j == 0), stop=(j == 7))
                ot = opool.tile([128, 512], dt.float32, tag="osb", name="osb")
                nc.vector.scalar_tensor_tensor(
                    ot[:], ps[:], qm_c(qt), bo_rep[:, c],
                    op0=ALU.mult, op1=ALU.add)
                nc.sync.dma_start(aps["out"][qr, c], ot[:])


def get_nc():
    if "nc" not in _NC_CACHE:
        _NC_CACHE["nc"] = _build_nc()
    return _NC_CACHE["nc"]


def make_in_maps(q, x, q_mask, k_mask, Wq, bq, Wk, bk, Wv, bv, Wo, bo):
    """Host-side shard/layout prep. Returns in_maps for cores 0..7."""
    wq_b = Wq.astype(BF16)
    wk_b = Wk.astype(BF16)
    wv_b = Wv.astype(BF16)
    wo_b = Wo.astype(BF16)
    bv_r = bv.astype(BF16).reshape(1, D)
    bo_r = bo.astype(BF16).reshape(1, D)
    bq_p = bq.astype(np.float32).reshape(8, 128).T
    bk_p = bk.astype(np.float32).reshape(8, 128).T

    in_maps = []
    for c in range(NCORES):
        b, qh = c // 2, c % 2
        qs = slice(QS * qh, QS * (qh + 1))
        kbias = np.where(k_mask[b] != 0, 0.0, NEG).astype(np.float32)
        consts = np.empty((128, 28), np.float32)
        consts[:, 0:8] = bq_p
        consts[:, 8:16] = bk_p
        consts[:, 16:24] = kbias.reshape(8, 128).T
        consts[:, 24:28] = q_mask[b, qs].astype(np.float32).reshape(4, 128).T
        in_maps.append({
            "qT": np.ascontiguousarray(q[b, qs, :].T).astype(BF16),
            "xT": np.ascontiguousarray(x[b].T).astype(BF16),
            "Wq": wq_b, "Wk": wk_b, "Wv": wv_b, "Wo": wo_b,
            "consts": np.ascontiguousarray(consts),
            "bvr": bv_r, "bor": bo_r,
        })
    return in_maps


def kernel(q, x, q_mask, k_mask, Wq, bq, Wk, bk, Wv, bv, Wo, bo):
    from concourse import bass_utils

    q = np.asarray(q, np.float32)
    x = np.asarray(x, np.float32)
    q_mask = np.asarray(q_mask)
    k_mask = np.asarray(k_mask)

    nc = get_nc()
    in_maps = make_in_maps(q, x, q_mask, k_mask, Wq, bq, Wk, bk, Wv, bv, Wo, bo)
    res = bass_utils.run_bass_kernel_spmd(nc, in_maps, core_ids=list(range(NCORES)))

    out = np.empty((B, LQ, D), np.float32)
    for c in range(NCORES):
        b, qh = c // 2, c % 2
        out[b, QS * qh:QS * (qh + 1), :] = res.results[c]["out"]
    return out



# revision 4
# speedup vs baseline: 1.4390x; 1.4390x over previous
"""Multi-head attention (B=4, L=1024, D=1024, H=16) on 8 TRN2 NeuronCores.

Sharding: (batch, vd-half) — core c handles batch c//2 and value/head
dimension half c%2 (heads 8*(c%2) .. 8*(c%2)+7). Each core computes its
512-wide slice of the Q/K/V projections for ALL 1024 queries/keys (no
duplicated projection work), full attention for its 8 heads, and a partial
output projection out_part = (O/denom * q_mask) @ Wo[vd_half].  The host
sums the two partials per batch and adds bo.  No collectives.

On-device layouts (all transposed so no transposes are needed):
  Q^T[vd, q] = Wq_h(lhsT) @ qT(rhs)  (+bq)   kt-outer: streams behind DMA
  K^T[vd, k] = Wk_h(lhsT) @ xT(rhs)  (+bk)   kt-outer
  V  [k, vd] = xT(lhsT) @ Wv_h(rhs)  (+bv via K=1 ones matmul)
  S^T[k, q]  = K^T_h(lhsT, K=64) @ Q^T_h for a head PAIR, row-packed into
               one [128, 2, 512] PSUM tile (concurrent via row groups)
  expS       = exp(S^T/8 + kmask_bias)   (ScalarE, PSUM->SBUF bf16)
  O^T+denom  = V_aug(lhsT, M=65) @ expS  (V cols + ones col per head)
  scale      = DVE cast denom rows -> PE K=1 broadcast of RAW denom ->
               wide reciprocal_approx_fast [128,512] -> fused multiply
  out_part[q, d] = (O^T_scaled.T @ Wo_h) * q_mask    (DVE epilogue, bf16)
"""

import os

os.environ.setdefault("MYCRO_LOCAL_CACHE", "1")

import numpy as np
import ml_dtypes

BF16 = ml_dtypes.bfloat16

B, LQ, LK = 4, 1024, 1024
D = 1024          # QD = KD = VD
H, DH = 16, 64
VH = 512          # vd half per core
NJ = 4            # vd-tiles (head pairs) per core
NCORES = 8
NEG = -1e4        # additive key-mask bias

_NC_CACHE = {}


def _build_nc():
    import concourse.bacc as bacc
    import concourse.mybir as mybir
    import concourse.tile as tile

    dt = mybir.dt

    nc = bacc.Bacc(
        "TRN2",
        debug=False,
        target_bir_lowering=False,
        num_devices=NCORES,
    )

    def din(name, shape, dtype):
        return nc.dram_tensor(name, shape, dtype, kind="ExternalInput").ap()

    aps = {
        "qT": din("qT", [D, LQ], dt.bfloat16),
        "xT": din("xT", [D, LK], dt.bfloat16),
        "Wq": din("Wq", [D, VH], dt.bfloat16),
        "Wk": din("Wk", [D, VH], dt.bfloat16),
        "Wv": din("Wv", [D, VH], dt.bfloat16),
        "Wo": din("Wo", [VH, D], dt.bfloat16),
        # packed per-partition constants: cols 0-3 bq, 4-7 bk, 8-15 kbias,
        # 16-23 q_mask (by query tile)
        "consts": din("consts", [128, 24], dt.float32),
        "bvr": din("bvr", [1, VH], dt.bfloat16),
        "out": nc.dram_tensor("out", [LQ, D], dt.bfloat16,
                              kind="ExternalOutput").ap(),
    }

    with tile.TileContext(nc) as tc:
        _body(tc, dt, mybir, aps)

    nc.compile()
    return nc


def _body(tc, dt, mybir, aps):
    from contextlib import ExitStack

    ALU = mybir.AluOpType
    AF = mybir.ActivationFunctionType
    nc = tc.nc
    with ExitStack() as ctx:
        const = ctx.enter_context(tc.tile_pool(name="const", bufs=1))
        espool = ctx.enter_context(tc.tile_pool(name="es", bufs=10))

        def ctile(shape, dtype, tag):
            return const.tile(shape, dtype, tag=tag, name=tag)

        # ---- input DMAs: kt-granular, ordered by first use, 3 queues ----
        cst = ctile([128, 24], dt.float32, "cst")
        nc.sync.dma_start(cst[:], aps["consts"][:, :])
        bv_sb = ctile([1, VH], dt.bfloat16, "bv")
        nc.scalar.dma_start(bv_sb[:], aps["bvr"][:, :])

        engs = [nc.sync, nc.scalar, nc.gpsimd]
        ei = [0]
        tiles = {}

        def load(name, dram, nfree, kt):
            tl = ctile([128, nfree], dt.bfloat16, f"{name}_{kt}")
            view = dram.rearrange("(t p) n -> p t n", p=128)
            engs[ei[0] % 3].dma_start(tl[:, :], view[:, kt, :])
            ei[0] += 1
            tiles[(name, kt)] = tl

        # qT/Wq interleaved (Q proj streams right behind these)
        for kt in range(8):
            load("qT", aps["qT"], LQ, kt)
            load("wq", aps["Wq"], VH, kt)
        for kt in range(8):
            load("xT", aps["xT"], LK, kt)
            load("wk", aps["Wk"], VH, kt)
        for kt in range(8):
            load("wv", aps["Wv"], VH, kt)
        for j in range(4):
            load("wo", aps["Wo"], D, j)

        t_of = lambda name, kt: tiles[(name, kt)]

        bq_c = lambda j: cst[:, j:j + 1]
        bk_c = lambda j: cst[:, 4 + j:5 + j]
        kb_c = lambda kt: cst[:, 8 + kt:9 + kt]
        qm_c = lambda qt: cst[:, 16 + qt:17 + qt]

        ones1 = ctile([1, 128], dt.bfloat16, "ones1")
        nc.gpsimd.memset(ones1[:], 1.0)
        ones64 = ctile([1, 64], dt.bfloat16, "ones64")
        nc.gpsimd.memset(ones64[:], 1.0)

        qTp = [ctile([128, LQ], dt.bfloat16, f"qTp{j}") for j in range(NJ)]
        kT_sb = [ctile([128, LK], dt.bfloat16, f"kT{j}") for j in range(NJ)]

        # ---- Q^T then K^T projections, kt-outer (stream behind DMA,
        # 8 PSUM banks held; pool released before attention pools open) ----
        with tc.tile_pool(name="proj", bufs=1, space="PSUM") as proj:
            qps = {}
            for j in range(NJ):
                for n in range(2):
                    qps[(j, n)] = proj.tile([128, 512], dt.float32,
                                            tag=f"pj{j}{n}", name="pj")
            for kt in range(8):
                for j in range(NJ):
                    for n in range(2):
                        nc.tensor.matmul(
                            qps[(j, n)][:],
                            t_of("wq", kt)[:, 128 * j:128 * (j + 1)],
                            t_of("qT", kt)[:, 512 * n:512 * (n + 1)],
                            start=(kt == 0), stop=(kt == 7))
            for j in range(NJ):
                for n in range(2):
                    nc.vector.tensor_scalar_add(
                        qTp[j][:, 512 * n:512 * (n + 1)], qps[(j, n)][:],
                        bq_c(j))

            kps = {}
            for j in range(NJ):
                for n in range(2):
                    kps[(j, n)] = proj.tile([128, 512], dt.float32,
                                            tag=f"pj{j}{n}", name="pj")
            for kd in range(8):
                for j in range(NJ):
                    for n in range(2):
                        nc.tensor.matmul(
                            kps[(j, n)][:],
                            t_of("wk", kd)[:, 128 * j:128 * (j + 1)],
                            t_of("xT", kd)[:, 512 * n:512 * (n + 1)],
                            start=(kd == 0), stop=(kd == 7))
            for j in range(NJ):
                for n in range(2):
                    nc.vector.tensor_scalar_add(
                        kT_sb[j][:, 512 * n:512 * (n + 1)], kps[(j, n)][:],
                        bk_c(j))

        # ---- attention + V-proj + out-proj, software-pipelined ----
        psum = ctx.enter_context(tc.tile_pool(name="psum", bufs=3, space="PSUM"))
        spair = ctx.enter_context(tc.tile_pool(name="spair", bufs=2, space="PSUM"))
        srp = ctx.enter_context(tc.tile_pool(name="srp", bufs=1, space="PSUM"))
        opool = ctx.enter_context(tc.tile_pool(name="osb", bufs=3))
        utp = ctx.enter_context(tc.tile_pool(name="utp", bufs=2))

        # V_aug tiles: per k-tile [128, 8*(64+1)]; local head h at cols
        # [65h, 65h+64), ones at 65h+64
        v_sb = [ctile([128, 8 * (DH + 1)], dt.bfloat16, f"v{t}")
                for t in range(8)]
        for t in range(8):
            ones_cols = v_sb[t][:].rearrange(
                "p (h c) -> p h c", c=DH + 1)[:, :, DH:DH + 1]
            nc.gpsimd.memset(ones_cols, 1.0)

        def v_proj(t):
            ps = psum.tile([128, 512], dt.float32, tag="ps", name="ps")
            for kd in range(8):
                nc.tensor.matmul(
                    ps[:], t_of("xT", kd)[:, 128 * t:128 * (t + 1)],
                    t_of("wv", kd)[:], start=(kd == 0), stop=False)
            nc.tensor.matmul(ps[:], ones1[:], bv_sb[:], start=False, stop=True)
            vout = v_sb[t][:].rearrange(
                "p (h c) -> p h c", c=DH + 1)[:, :, 0:DH]
            vin = ps[:].rearrange("p (i c) -> p i c", c=DH)
            nc.vector.tensor_copy(vout, vin)

        # stage order: qh-outer, j-inner.  stage si -> (qh, j)
        STAGES = [(qh, j) for qh in range(2) for j in range(NJ)]
        oTs = [ctile([128, LQ], dt.bfloat16, f"oTs{j}") for j in range(NJ)]
        dnb = ctile([1, 2, 512], dt.bfloat16, "dnb")
        rsr = ctile([128, 512], dt.float32, "rsr")

        es_tiles = {}

        def s_stage(si, kt):
            qh, j = STAGES[si]
            qc = slice(512 * qh, 512 * (qh + 1))
            kc = slice(128 * kt, 128 * (kt + 1))
            sp = spair.tile([128, 2, 512], dt.float32, tag="sp", name="sp")
            nc.tensor.matmul(sp[:, 0, :], kT_sb[j][0:64, kc],
                             qTp[j][0:64, qc], start=True, stop=True)
            nc.tensor.matmul(sp[:, 1, :], kT_sb[j][64:128, kc],
                             qTp[j][64:128, qc], start=True, stop=True)
            es = espool.tile([128, 2, 512], dt.bfloat16, tag="es", name="es")
            nc.scalar.activation(es[:], sp[:], AF.Exp,
                                 bias=kb_c(kt), scale=0.125)
            es_tiles[(si, kt)] = es

        def o_stage(si, kt, oA, oB):
            qh, j = STAGES[si]
            hA, hB = 2 * j, 2 * j + 1
            es = es_tiles.pop((si, kt))
            nc.tensor.matmul(oA[0:65, :], v_sb[kt][:, 65 * hA:65 * hA + 65],
                             es[:, 0, :], start=(kt == 0), stop=(kt == 7))
            nc.tensor.matmul(oB[0:65, :], v_sb[kt][:, 65 * hB:65 * hB + 65],
                             es[:, 1, :], start=(kt == 0), stop=(kt == 7))

        # flat S/exp stream runs LOOKAHEAD stages ahead of O accumulation
        LOOKAHEAD = 8
        flat = [(si, kt) for si in range(8) for kt in range(8)]
        s_cursor = [0]

        def advance_s(upto):
            while s_cursor[0] < min(upto, 64):
                s_stage(*flat[s_cursor[0]])
                s_cursor[0] += 1

        # out-projection: tile (qt, n); qt 0-3 from qh0, 4-7 from qh1
        def out_tile(qt, n):
            qr = slice(128 * qt, 128 * (qt + 1))
            c = slice(512 * n, 512 * (n + 1))
            ps = psum.tile([128, 512], dt.float32, tag="ps", name="ps")
            for j in range(NJ):
                nc.tensor.matmul(ps[:], oTs[j][:, qr], t_of("wo", j)[:, c],
                                 start=(j == 0), stop=(j == NJ - 1))
            ot = opool.tile([128, 512], dt.bfloat16, tag="osb", name="osb")
            nc.vector.tensor_scalar_mul(ot[:], ps[:], qm_c(qt))
            nc.sync.dma_start(aps["out"][qr, c], ot[:])

        # fillers: out-proj qh0 tiles, injectable from stage 4 on
        fillers = []
        fcursor = [0]

        def run_filler(k=1):
            for _ in range(k):
                if fcursor[0] < len(fillers):
                    fillers[fcursor[0]]()
                    fcursor[0] += 1

        def o_alloc():
            oA = psum.tile([128, 512], dt.float32, tag="ps", name="ps")
            oB = psum.tile([128, 512], dt.float32, tag="ps", name="ps")
            return oA, oB

        # prologue: start S/exp stream, then first V tiles between S stages
        advance_s(2)
        v_proj(0)
        advance_s(4)
        v_proj(1)
        advance_s(6)
        v_proj(2)

        cur = o_alloc()
        for si in range(8):
            qh, j = STAGES[si]
            if si == 4:
                for qt in range(4):
                    for n in range(2):
                        fillers.append(lambda qt=qt, n=n: out_tile(qt, n))
            oA, oB = cur
            for kt in range(8):
                advance_s(8 * si + kt + 1 + LOOKAHEAD)
                o_stage(si, kt, oA, oB)
                if si == 0 and 2 <= kt < 7:
                    v_proj(kt + 1)
                elif kt in (2, 5):
                    run_filler()
            if si < 7:
                cur = o_alloc()

            # scale chain: cast raw denom rows -> broadcast -> wide recip
            nc.vector.tensor_copy(dnb[:, 0, :], oA[64:65, :])
            nc.vector.tensor_copy(dnb[:, 1, :], oB[64:65, :])
            oTu = utp.tile([128, 512], dt.bfloat16, tag="oTu", name="oTu")
            nc.vector.tensor_copy(oTu[0:64, :], oA[0:64, :])
            nc.vector.tensor_copy(oTu[64:128, :], oB[0:64, :])
            sr = srp.tile([128, 512], dt.float32, tag="sr", name="sr")
            nc.tensor.matmul(sr[0:64, :], ones64[:], dnb[:, 0, :],
                             start=True, stop=True)
            nc.tensor.matmul(sr[64:128, :], ones64[:], dnb[:, 1, :],
                             start=True, stop=True, tile_position=(0, 64))
            run_filler()
            nc.vector.reciprocal_approx_fast(out=rsr[:], in_=sr[:])
            nc.vector.scalar_tensor_tensor(
                oTs[j][:, 512 * qh:512 * (qh + 1)], oTu[:], 0.0, rsr[:],
                op0=ALU.bypass, op1=ALU.mult)

        run_filler(len(fillers))
        # qh1 out tiles
        for qt in range(4, 8):
            for n in range(2):
                out_tile(qt, n)


def get_nc():
    if "nc" not in _NC_CACHE:
        _NC_CACHE["nc"] = _build_nc()
    return _NC_CACHE["nc"]


def make_in_maps(q, x, q_mask, k_mask, Wq, bq, Wk, bk, Wv, bv, Wo, bo):
    """Host-side shard/layout prep. Returns in_maps for cores 0..7."""
    in_maps = []
    for c in range(NCORES):
        b, hf = c // 2, c % 2
        vs = slice(VH * hf, VH * (hf + 1))
        kbias = np.where(k_mask[b] != 0, 0.0, NEG).astype(np.float32)
        consts = np.empty((128, 24), np.float32)
        consts[:, 0:4] = np.asarray(bq, np.float32)[vs].reshape(4, 128).T
        consts[:, 4:8] = np.asarray(bk, np.float32)[vs].reshape(4, 128).T
        consts[:, 8:16] = kbias.reshape(8, 128).T
        consts[:, 16:24] = q_mask[b].astype(np.float32).reshape(8, 128).T
        in_maps.append({
            "qT": np.ascontiguousarray(q[b].T).astype(BF16),
            "xT": np.ascontiguousarray(x[b].T).astype(BF16),
            "Wq": np.ascontiguousarray(np.asarray(Wq)[:, vs]).astype(BF16),
            "Wk": np.ascontiguousarray(np.asarray(Wk)[:, vs]).astype(BF16),
            "Wv": np.ascontiguousarray(np.asarray(Wv)[:, vs]).astype(BF16),
            "Wo": np.ascontiguousarray(np.asarray(Wo)[vs, :]).astype(BF16),
            "consts": np.ascontiguousarray(consts),
            "bvr": np.asarray(bv, np.float32)[vs].astype(BF16).reshape(1, VH),
        })
    return in_maps


def combine_outputs(results, bo):
    """Sum the two vd-half partials per batch and add bo."""
    out = np.empty((B, LQ, D), np.float32)
    for b in range(B):
        out[b] = (results[2 * b]["out"].astype(np.float32)
                  + results[2 * b + 1]["out"].astype(np.float32)
                  + bo[None, :])
    return out


def kernel(q, x, q_mask, k_mask, Wq, bq, Wk, bk, Wv, bv, Wo, bo):
    from concourse import bass_utils

    q = np.asarray(q, np.float32)
    x = np.asarray(x, np.float32)
    q_mask = np.asarray(q_mask)
    k_mask = np.asarray(k_mask)

    nc = get_nc()
    in_maps = make_in_maps(q, x, q_mask, k_mask, Wq, bq, Wk, bk, Wv, bv, Wo, bo)
    res = bass_utils.run_bass_kernel_spmd(nc, in_maps, core_ids=list(range(NCORES)))
    return combine_outputs(res.results, np.asarray(bo, np.float32))
